# revision 1
# baseline (speedup 1.0000x reference)
"""Trainium2 Bass kernel for nn_MultiHeadAttention (B=2, S=2048, E=1024, H=16, D=64).

Sharding: 8 NeuronCores = 2 batches x 4 head-groups (data + tensor parallel).
Each core computes, for its batch b and its 4 heads: q/k/v projections
(transposed layout), attention with softmax in transposed-score space (sums
via an extra ones-column appended to V), and its partial of the output
projection.  Host sums the 4 partials per batch and adds the output bias.

Per-core schedule:
  phase 1 (PE/DMA-bound): projection matmuls, evacuated from PSUM by the
    (idle) Activation engine with fused bias add; V tiles transposed on PE.
    DMA issue order is tuned so each chunk's x slice and weights land just
    before PE needs them (DMA transfers serialize on the shared engines).
  phase 2 (ACT-bound): per 256-token q-region and head-pair, scores for a
    k-tile PAIR -> one exp (ACT) -> attnV accumulate, with attnV lagged one
    pair so PE never waits on exp; softmax normalization runs on DVE+Pool
    off the critical path; the output projection of region r-1 is
    interleaved into region r.

All matmuls run as float32r (fp32 data, bf16-rate PE path, fp32 accumulate).
"""

import time

import numpy as np

import concourse.mybir as mybir
import concourse.tile as tile
from concourse import bacc
from concourse.bass_utils import run_bass_kernel_spmd
from concourse.masks import make_identity

F32 = mybir.dt.float32
F32R = mybir.dt.float32r
AF = mybir.ActivationFunctionType

# Problem shapes (hardcoded per contest contract)
B, S, E, H, D = 2, 2048, 1024, 16, 64
NCORES = 8
HPC = H // (NCORES // B)   # heads per core = 4
NP = HPC // 2              # head-pairs per core = 2
DH = HPC * D               # head dims per core = 256
P = 128                    # partitions
SC = 512                   # projection chunk (fp32 max moving)
KO = E // P                # contraction tiles for projections = 8
NSC = S // SC              # token chunks for projections = 4
KI = S // P                # k tiles = 16
KP = KI // 2               # k-tile pairs = 8
RSC = 256                  # attention q-region width
NR = S // RSC              # q regions = 8
NG = 3 * DH // P           # projection column groups = 6
VW = 2 * (D + 1)           # v_aug row width for 2 heads = 130


def build_kernel(tc, xt, wqkvt, bqkv, wot, partial):
    nc = tc.nc
    xt_r = xt.rearrange("(ko p) s -> p ko s", p=P)          # [128, 8, 2048]
    wqkvt_r = wqkvt.rearrange("(ko p) n -> p ko n", p=P)    # [128, 8, 768]

    with (
        tc.tile_pool(name="persist", bufs=1) as persist,
        tc.tile_pool(name="stream", bufs=3) as stream,
        tc.tile_pool(name="small", bufs=2) as small,
    ):
        # ---- resident tensors / startup DMA order ----
        # Transfers serialize on the DMA engines, so issue in the order PE
        # will consume: bias, chunk-0 x (per-ko slices so the first matmuls
        # chase them), q weights early (between the first two slices), then
        # chunk-1 x, k weights, v weights, chunks 2-3, Wo.
        xts = [stream.tile([P, KO, SC], F32R, name="xt", tag="xt", bufs=3)
               for _ in range(2)]
        wqkv_sb = persist.tile([P, KO, 3 * DH], F32R)       # [128, 8, 768]
        bias_sb = persist.tile([P, NG], F32)
        nc.sync.dma_start(bias_sb[:], bqkv.rearrange("(g p) o -> p (g o)", p=P))
        for ko in range(KO):
            nc.sync.dma_start(xts[0][:, ko, :], xt_r[:, ko, 0:SC])
            nc.sync.dma_start(wqkv_sb[:, ko, :], wqkvt_r[:, ko, :])
        for ko in range(KO):
            nc.sync.dma_start(xts[1][:, ko, :], xt_r[:, ko, SC:2 * SC])

        ident = persist.tile([P, P], F32)
        make_identity(nc, ident[:])

        qT_sb = persist.tile([P, NP, S], F32R)   # per pair: rows = 2 heads x 64
        kT_sb = persist.tile([P, NP, S], F32R)
        v_sb = persist.tile([P, NP * KI, VW], F32R)  # per k-tile: [1 vA | 1 vB]
        attnT_sb = persist.tile([P, NP, S], F32R)
        wot_sb = persist.tile([P, NP, E], F32R)

        # ones columns for the softmax denominator (lands on PSUM partition 0)
        for col in (0, D + 1):
            nc.scalar.activation(
                v_sb[:, :, col],
                ident[:, 0:NP * KI],
                AF.Identity, bias=1.0, scale=0.0)

        # ---- phase 1: q/k/v projections + V transposes ----
        with (
            tc.tile_pool(name="psp", bufs=3, space="PSUM") as psp,
            tc.tile_pool(name="pstr", bufs=2, space="PSUM") as pstr,
        ):
            for c in range(NSC):
                cs = slice(c * SC, (c + 1) * SC)
                if c < 2:
                    xt_t = xts[c]
                else:
                    xt_t = stream.tile([P, KO, SC], F32R, name="xt",
                                       tag="xt", bufs=3)
                    nc.sync.dma_start(xt_t[:], xt_r[:, :, cs])
                if c == 1:
                    # Wo loads late, after the startup DMA crunch
                    nc.sync.dma_start(
                        wot_sb[:], wot.rearrange("(j p) e -> p j e", p=P))
                vT_c = small.tile([P, NP, SC], F32, tag="vtc")
                for g in range(NG):
                    ps = psp.tile([P, SC], F32, tag="proj")
                    for ko in range(KO):
                        nc.tensor.matmul(
                            ps[:],
                            wqkv_sb[:, ko, g * P:(g + 1) * P],
                            xt_t[:, ko, :],
                            start=(ko == 0), stop=(ko == KO - 1),
                        )
                    if g < 2:
                        dest = qT_sb[:, g, cs]
                    elif g < 4:
                        dest = kT_sb[:, g - 2, cs]
                    else:
                        dest = vT_c[:, g - 4, :]
                    # evacuate + bias add on the otherwise-idle ACT engine
                    nc.scalar.activation(dest, ps[:], AF.Identity,
                                         bias=bias_sb[:, g:g + 1], scale=1.0)
                # transpose this chunk's V tiles (PE) and pack into v_sb (DVE)
                for p in range(NP):
                    for t in range(SC // P):
                        pt = pstr.tile([P, P], F32, tag="tr")
                        nc.tensor.transpose(pt[:], vT_c[:, p, t * P:(t + 1) * P],
                                            ident[:])
                        dst = v_sb[:, p * KI + c * (SC // P) + t, :].rearrange(
                            "p (h w) -> p h w", h=2)[:, :, 1:D + 1]
                        nc.vector.tensor_copy(
                            dst, pt[:].rearrange("p (h d) -> p h d", h=2))

        # ---- phase 2: byte-for-byte baseline shapes (SC=512) ----
        with (
            tc.tile_pool(name="pssc", bufs=2, space="PSUM") as pssc,
            tc.tile_pool(name="psoa", bufs=1, space="PSUM") as psoa,
            tc.tile_pool(name="pspo", bufs=2, space="PSUM") as pspo,
        ):
            for r in range(S // SC):
                rs = slice(r * SC, (r + 1) * SC)
                for p in range(NP):
                    # single-buffered accumulators are safe with the
                    # lag-2 pipeline (next unit's first attnV arrives well
                    # after the evacuation copies); the freed bank double-
                    # buffers the out-projection instead
                    po = [psoa.tile([P, SC], F32, name=f"oa{h}", tag=f"oa{h}",
                                    bufs=1)
                          for h in range(2)]
                    exs = {}

                    # attnV lags scores by two k-tiles: PE never sits in the
                    # exp -> attnV -> next-scores semaphore chain, so the ACT
                    # engine (the attention-phase bottleneck) stays dense
                    def attn_v(k):
                        ex_k = exs.pop(k)
                        for h in range(2):
                            nc.tensor.matmul(
                                po[h][0:D + 1, :],
                                v_sb[:, p * KI + k,
                                     h * (D + 1):(h + 1) * (D + 1)],
                                ex_k[:, h, :],
                                start=(k == 0), stop=(k == KI - 1),
                                skip_group_check=True,
                            )

                    for ki in range(KI):
                        pss = pssc.tile([P, 2, SC], F32, tag="sc")
                        for h in range(2):
                            hr = slice(h * D, (h + 1) * D)
                            nc.tensor.matmul(
                                pss[:, h, :],
                                kT_sb[hr, p, ki * P:(ki + 1) * P],
                                qT_sb[hr, p, rs],
                                start=True, stop=True,
                            )
                        ex = stream.tile([P, 2, SC], F32R, tag="exp", bufs=4)
                        nc.scalar.activation(ex[:], pss[:], AF.Exp,
                                             scale=1.0 / np.sqrt(D))
                        exs[ki] = ex
                        if ki >= 2:
                            attn_v(ki - 2)
                    attn_v(KI - 2)
                    attn_v(KI - 1)
                    for h in range(2):
                        oa = small.tile([D + 1, SC], F32, tag="oa_sb")
                        nc.vector.tensor_copy(oa[:], po[h][0:D + 1, :])
                        recip = small.tile([1, SC], F32, tag="recip")
                        nc.vector.reciprocal(recip[:], oa[0:1, :])
                        bc = small.tile([D + 1, SC], F32, tag="bc")
                        nc.gpsimd.partition_broadcast(bc[:], recip[:])
                        nrm = small.tile([D + 1, SC], F32R, tag="nrm")
                        nc.vector.tensor_mul(nrm[:], oa[:], bc[:])
                        nc.sync.dma_start(attnT_sb[h * D:(h + 1) * D, p, rs],
                                          nrm[1:D + 1, :])
                for sti in range(SC // P):
                    row = r * SC + sti * P
                    ot = stream.tile([P, E], F32, tag="ot")
                    for e in range(E // SC):
                        pp = pspo.tile([P, SC], F32, tag="po")
                        for p2 in range(NP):
                            nc.tensor.matmul(
                                pp[:],
                                attnT_sb[:, p2, row:row + P],
                                wot_sb[:, p2, e * SC:(e + 1) * SC],
                                start=(p2 == 0), stop=(p2 == NP - 1),
                            )
                        nc.vector.tensor_copy(ot[:, e * SC:(e + 1) * SC],
                                              pp[:])
                    nc.sync.dma_start(partial[row:row + P, :], ot[:])


def build_module():
    nc = bacc.Bacc("TRN2", target_bir_lowering=False, debug=False,
                   num_devices=NCORES)
    xt = nc.dram_tensor("xt", [E, S], F32R, kind="ExternalInput").ap()
    wqkvt = nc.dram_tensor("wqkvt", [E, 3 * DH], F32R, kind="ExternalInput").ap()
    bqkv = nc.dram_tensor("bqkv", [3 * DH, 1], F32, kind="ExternalInput").ap()
    wot = nc.dram_tensor("wot", [DH, E], F32R, kind="ExternalInput").ap()
    partial = nc.dram_tensor("partial", [S, E], F32, kind="ExternalOutput").ap()
    with tile.TileContext(nc) as tc:
        build_kernel(tc, xt, wqkvt, bqkv, wot, partial)
    nc.compile()
    return nc


def make_in_maps(x, Wq, bq, Wk, bk, Wv, bv, Wo, bo):
    xts = [np.ascontiguousarray(x[b].T).astype(np.float32) for b in range(B)]
    in_maps = []
    for c in range(NCORES):
        b, hg = divmod(c, NCORES // B)
        rows = slice(hg * DH, (hg + 1) * DH)
        wqkvt = np.ascontiguousarray(
            np.concatenate([Wq[rows], Wk[rows], Wv[rows]], axis=0).T
        ).astype(np.float32)
        bqkv = np.concatenate([bq[rows], bk[rows], bv[rows]]).reshape(3 * DH, 1)
        wot = np.ascontiguousarray(Wo[:, rows].T).astype(np.float32)
        in_maps.append({
            "xt": xts[b],
            "wqkvt": wqkvt,
            "bqkv": bqkv.astype(np.float32),
            "wot": wot,
        })
    return in_maps


_NC_CACHE = None


def kernel(x, Wq, bq, Wk, bk, Wv, bv, Wo, bo, _trace=False):
    global _NC_CACHE
    x = np.asarray(x)
    if _NC_CACHE is None:
        _NC_CACHE = build_module()
    nc = _NC_CACHE
    in_maps = make_in_maps(np.asarray(x), np.asarray(Wq), np.asarray(bq),
                           np.asarray(Wk), np.asarray(bk), np.asarray(Wv),
                           np.asarray(bv), np.asarray(Wo), np.asarray(bo))
    # transient NRT_EXEC_UNIT_UNRECOVERABLE flakes have been observed on this
    # fabric; a short-delay retry has always succeeded.
    last_err = None
    for attempt in range(3):
        try:
            res = run_bass_kernel_spmd(nc, in_maps, core_ids=list(range(NCORES)),
                                       trace=_trace)
            break
        except Exception as e:  # noqa: BLE001
            last_err = e
            time.sleep(10 * (attempt + 1))
    else:
        raise last_err
    partials = np.stack([res.results[c]["partial"] for c in range(NCORES)])
    out = (partials.reshape(B, NCORES // B, S, E).sum(axis=1, dtype=np.float64)
           + np.asarray(bo, dtype=np.float64))
    out = out.astype(np.float32).reshape(B, S, E)
    if _trace:
        return out, res
    return out



# revision 2
# speedup vs baseline: 13.4936x; 13.4936x over previous
"""Trainium2 Bass kernel for nn_MultiHeadAttention (B=2, S=2048, E=1024, H=16, D=64).

Sharding: 8 NeuronCores = 2 batches x 4 head-groups (data + tensor parallel).
Each core computes, for its batch b and its 4 heads: q/k/v projections
(transposed layout), attention with softmax in transposed-score space (sums
via an extra ones-column appended to V), and its partial of the output
projection.  Host sums the 4 partials per batch and adds the output bias.

Per-core schedule:
  phase 1 (PE/DMA-bound): projection matmuls, evacuated from PSUM by the
    (idle) Activation engine with fused bias add; V tiles transposed on PE.
    DMA issue order is tuned so each chunk's x slice and weights land just
    before PE needs them (DMA transfers serialize on the shared engines).
  phase 2 (ACT-bound): per 256-token q-region and head-pair, scores for a
    k-tile PAIR -> one exp (ACT) -> attnV accumulate, with attnV lagged one
    pair so PE never waits on exp; softmax normalization runs on DVE+Pool
    off the critical path; the output projection of region r-1 is
    interleaved into region r.

All matmuls run as float32r (fp32 data, bf16-rate PE path, fp32 accumulate).
"""

import time

import numpy as np

import concourse.mybir as mybir
import concourse.tile as tile
from concourse import bacc
from concourse.bass_utils import run_bass_kernel_spmd
from concourse.masks import make_identity

F32 = mybir.dt.float32
F32R = mybir.dt.float32r
AF = mybir.ActivationFunctionType

# Problem shapes (hardcoded per contest contract)
B, S, E, H, D = 2, 2048, 1024, 16, 64
NCORES = 8
HPC = H // (NCORES // B)   # heads per core = 4
NP = HPC // 2              # head-pairs per core = 2
DH = HPC * D               # head dims per core = 256
P = 128                    # partitions
SC = 512                   # projection chunk (fp32 max moving)
KO = E // P                # contraction tiles for projections = 8
NSC = S // SC              # token chunks for projections = 4
KI = S // P                # k tiles = 16
KP = KI // 2               # k-tile pairs = 8
RSC = 256                  # attention q-region width
NR = S // RSC              # q regions = 8
NG = 3 * DH // P           # projection column groups = 6
VW = 2 * (D + 1)           # v_aug row width for 2 heads = 130


def build_kernel(tc, xt, wqkvt, bqkv, wot, partial):
    nc = tc.nc
    xt_r = xt.rearrange("(ko p) s -> p ko s", p=P)          # [128, 8, 2048]
    wqkvt_r = wqkvt.rearrange("(ko p) n -> p ko n", p=P)    # [128, 8, 768]

    with (
        tc.tile_pool(name="persist", bufs=1) as persist,
        tc.tile_pool(name="stream", bufs=3) as stream,
        tc.tile_pool(name="small", bufs=2) as small,
    ):
        # ---- resident tensors / startup DMA order ----
        # Transfers serialize on the DMA engines, so issue in the order PE
        # will consume: bias, chunk-0 x (per-ko slices so the first matmuls
        # chase them), q weights early (between the first two slices), then
        # chunk-1 x, k weights, v weights, chunks 2-3, Wo.
        xts = [stream.tile([P, KO, SC], F32R, name="xt", tag="xt", bufs=3)
               for _ in range(2)]
        wqkv_sb = persist.tile([P, KO, 3 * DH], F32R)       # [128, 8, 768]
        bias_sb = persist.tile([P, NG], F32)
        nc.sync.dma_start(bias_sb[:], bqkv.rearrange("(g p) o -> p (g o)", p=P))
        for ko in range(KO):
            nc.sync.dma_start(xts[0][:, ko, :], xt_r[:, ko, 0:SC])
            nc.sync.dma_start(wqkv_sb[:, ko, :], wqkvt_r[:, ko, :])
        for ko in range(KO):
            nc.sync.dma_start(xts[1][:, ko, :], xt_r[:, ko, SC:2 * SC])

        ident = persist.tile([P, P], F32)
        make_identity(nc, ident[:])

        qT_sb = persist.tile([P, NP, S], F32R)   # per pair: rows = 2 heads x 64
        kT_sb = persist.tile([P, NP, S], F32R)
        v_sb = persist.tile([P, NP * KI, VW], F32R)  # per k-tile: [1 vA | 1 vB]
        attnT_sb = persist.tile([P, NP, S], F32R)
        wot_sb = persist.tile([P, NP, E], F32R)

        # ones columns for the softmax denominator (lands on PSUM partition 0)
        for col in (0, D + 1):
            nc.scalar.activation(
                v_sb[:, :, col],
                ident[:, 0:NP * KI],
                AF.Identity, bias=1.0, scale=0.0)

        # ---- phase 1: q/k/v projections + V transposes ----
        with (
            tc.tile_pool(name="psp", bufs=3, space="PSUM") as psp,
            tc.tile_pool(name="pstr", bufs=2, space="PSUM") as pstr,
        ):
            for c in range(NSC):
                cs = slice(c * SC, (c + 1) * SC)
                if c < 2:
                    xt_t = xts[c]
                else:
                    xt_t = stream.tile([P, KO, SC], F32R, name="xt",
                                       tag="xt", bufs=3)
                    nc.sync.dma_start(xt_t[:], xt_r[:, :, cs])
                if c == 1:
                    # Wo loads late, after the startup DMA crunch
                    nc.sync.dma_start(
                        wot_sb[:], wot.rearrange("(j p) e -> p j e", p=P))
                vT_c = small.tile([P, NP, SC], F32, tag="vtc")
                for g in range(NG):
                    ps = psp.tile([P, SC], F32, tag="proj")
                    for ko in range(KO):
                        nc.tensor.matmul(
                            ps[:],
                            wqkv_sb[:, ko, g * P:(g + 1) * P],
                            xt_t[:, ko, :],
                            start=(ko == 0), stop=(ko == KO - 1),
                        )
                    if g < 2:
                        dest = qT_sb[:, g, cs]
                    elif g < 4:
                        dest = kT_sb[:, g - 2, cs]
                    else:
                        dest = vT_c[:, g - 4, :]
                    # evacuate + bias add on the otherwise-idle ACT engine
                    nc.scalar.activation(dest, ps[:], AF.Identity,
                                         bias=bias_sb[:, g:g + 1], scale=1.0)
                # transpose this chunk's V tiles (PE) and pack into v_sb (DVE)
                for p in range(NP):
                    for t in range(SC // P):
                        pt = pstr.tile([P, P], F32, tag="tr")
                        nc.tensor.transpose(pt[:], vT_c[:, p, t * P:(t + 1) * P],
                                            ident[:])
                        dst = v_sb[:, p * KI + c * (SC // P) + t, :].rearrange(
                            "p (h w) -> p h w", h=2)[:, :, 1:D + 1]
                        nc.vector.tensor_copy(
                            dst, pt[:].rearrange("p (h d) -> p h d", h=2))

        # ---- phase 2: byte-for-byte baseline shapes (SC=512) ----
        with (
            tc.tile_pool(name="pssc", bufs=2, space="PSUM") as pssc,
            tc.tile_pool(name="psoa", bufs=1, space="PSUM") as psoa,
            tc.tile_pool(name="pspo", bufs=2, space="PSUM") as pspo,
        ):
            for r in range(S // SC):
                rs = slice(r * SC, (r + 1) * SC)
                for p in range(NP):
                    # single-buffered accumulators are safe with the
                    # lag-2 pipeline (next unit's first attnV arrives well
                    # after the evacuation copies); the freed bank double-
                    # buffers the out-projection instead
                    po = [psoa.tile([P, SC], F32, name=f"oa{h}", tag=f"oa{h}",
                                    bufs=1)
                          for h in range(2)]
                    exs = {}

                    # attnV lags scores by two k-tiles: PE never sits in the
                    # exp -> attnV -> next-scores semaphore chain, so the ACT
                    # engine (the attention-phase bottleneck) stays dense
                    def attn_v(k):
                        ex_k = exs.pop(k)
                        for h in range(2):
                            nc.tensor.matmul(
                                po[h][0:D + 1, :],
                                v_sb[:, p * KI + k,
                                     h * (D + 1):(h + 1) * (D + 1)],
                                ex_k[:, h, :],
                                start=(k == 0), stop=(k == KI - 1),
                                skip_group_check=True,
                            )

                    for ki in range(KI):
                        pss = pssc.tile([P, 2, SC], F32, tag="sc")
                        for h in range(2):
                            hr = slice(h * D, (h + 1) * D)
                            nc.tensor.matmul(
                                pss[:, h, :],
                                kT_sb[hr, p, ki * P:(ki + 1) * P],
                                qT_sb[hr, p, rs],
                                start=True, stop=True,
                            )
                        ex = stream.tile([P, 2, SC], F32R, tag="exp", bufs=4)
                        nc.scalar.activation(ex[:], pss[:], AF.Exp,
                                             scale=1.0 / np.sqrt(D))
                        exs[ki] = ex
                        if ki >= 2:
                            attn_v(ki - 2)
                    attn_v(KI - 2)
                    attn_v(KI - 1)
                    for h in range(2):
                        oa = small.tile([D + 1, SC], F32, tag="oa_sb")
                        nc.vector.tensor_copy(oa[:], po[h][0:D + 1, :])
                        recip = small.tile([1, SC], F32, tag="recip")
                        nc.vector.reciprocal(recip[:], oa[0:1, :])
                        bc = small.tile([D + 1, SC], F32, tag="bc")
                        nc.gpsimd.partition_broadcast(bc[:], recip[:])
                        nrm = small.tile([D + 1, SC], F32R, tag="nrm")
                        nc.vector.tensor_mul(nrm[:], oa[:], bc[:])
                        nc.sync.dma_start(attnT_sb[h * D:(h + 1) * D, p, rs],
                                          nrm[1:D + 1, :])
                for sti in range(SC // P):
                    row = r * SC + sti * P
                    ot = stream.tile([P, E], F32, tag="ot")
                    for e in range(E // SC):
                        pp = pspo.tile([P, SC], F32, tag="po")
                        for p2 in range(NP):
                            nc.tensor.matmul(
                                pp[:],
                                attnT_sb[:, p2, row:row + P],
                                wot_sb[:, p2, e * SC:(e + 1) * SC],
                                start=(p2 == 0), stop=(p2 == NP - 1),
                            )
                        nc.vector.tensor_copy(ot[:, e * SC:(e + 1) * SC],
                                              pp[:])
                    nc.sync.dma_start(partial[row:row + P, :], ot[:])


def build_module(reps=1):
    """reps>1 replicates the kernel body (serialized by an all-engine
    barrier) inside one NEFF so per-kernel HW time can be measured as the
    slope vs reps — the per-execute dispatch floor cancels."""
    nc = bacc.Bacc("TRN2", target_bir_lowering=False, debug=False,
                   num_devices=NCORES)
    xt = nc.dram_tensor("xt", [E, S], F32R, kind="ExternalInput").ap()
    wqkvt = nc.dram_tensor("wqkvt", [E, 3 * DH], F32R, kind="ExternalInput").ap()
    bqkv = nc.dram_tensor("bqkv", [3 * DH, 1], F32, kind="ExternalInput").ap()
    wot = nc.dram_tensor("wot", [DH, E], F32R, kind="ExternalInput").ap()
    partial = nc.dram_tensor("partial", [S, E], F32, kind="ExternalOutput").ap()
    with tile.TileContext(nc) as tc:
        for r in range(reps):
            build_kernel(tc, xt, wqkvt, bqkv, wot, partial)
            if reps > 1 and r < reps - 1:
                tc.strict_bb_all_engine_barrier()
    nc.compile()
    return nc


def make_in_maps(x, Wq, bq, Wk, bk, Wv, bv, Wo, bo):
    xts = [np.ascontiguousarray(x[b].T).astype(np.float32) for b in range(B)]
    in_maps = []
    for c in range(NCORES):
        b, hg = divmod(c, NCORES // B)
        rows = slice(hg * DH, (hg + 1) * DH)
        wqkvt = np.ascontiguousarray(
            np.concatenate([Wq[rows], Wk[rows], Wv[rows]], axis=0).T
        ).astype(np.float32)
        bqkv = np.concatenate([bq[rows], bk[rows], bv[rows]]).reshape(3 * DH, 1)
        wot = np.ascontiguousarray(Wo[:, rows].T).astype(np.float32)
        in_maps.append({
            "xt": xts[b],
            "wqkvt": wqkvt,
            "bqkv": bqkv.astype(np.float32),
            "wot": wot,
        })
    return in_maps


_NC_CACHE = None


def kernel(x, Wq, bq, Wk, bk, Wv, bv, Wo, bo, _trace=False):
    global _NC_CACHE
    x = np.asarray(x)
    if _NC_CACHE is None:
        _NC_CACHE = build_module()
    nc = _NC_CACHE
    in_maps = make_in_maps(np.asarray(x), np.asarray(Wq), np.asarray(bq),
                           np.asarray(Wk), np.asarray(bk), np.asarray(Wv),
                           np.asarray(bv), np.asarray(Wo), np.asarray(bo))
    # transient NRT_EXEC_UNIT_UNRECOVERABLE flakes have been observed on this
    # fabric; a short-delay retry has always succeeded.
    last_err = None
    for attempt in range(3):
        try:
            res = run_bass_kernel_spmd(nc, in_maps, core_ids=list(range(NCORES)),
                                       trace=_trace)
            break
        except Exception as e:  # noqa: BLE001
            last_err = e
            time.sleep(10 * (attempt + 1))
    else:
        raise last_err
    partials = np.stack([res.results[c]["partial"] for c in range(NCORES)])
    out = (partials.reshape(B, NCORES // B, S, E).sum(axis=1, dtype=np.float64)
           + np.asarray(bo, dtype=np.float64))
    out = out.astype(np.float32).reshape(B, S, E)
    if _trace:
        return out, res
    return out



# revision 13
# speedup vs baseline: 16.4032x; 1.2156x over previous
"""Trainium2 Bass kernel for nn_MultiHeadAttention (B=2, S=2048, E=1024, H=16, D=64).

Sharding: 8 NeuronCores = 2 batches x 4 head-groups (data + tensor parallel).
Each core computes, for its batch b and its 4 heads: q/k/v projections,
attention with softmax in transposed-score space (denominator via an extra
ones-column appended to V), and its partial of the output projection.  Host
sums the 4 partials per batch and adds the output bias.

Schedule (single fused pipeline; PE is the binding engine at ~167us of
matmul work, ACT ~137us of exp hides under it):
  - DMA order: bias, x chunk0, Wk, Wq, x chunks 1-3, Wv, Wo.  K/Q
    projections chase the x chunks so kT/qT complete as soon as the input
    lands; exp (ACT) starts right after.
  - V is projected DIRECTLY in [token, dim] layout (stationary x-tile,
    moving Wv) into PSUM, so no PE transposes / DVE repacking are needed
    and all PSUM pools coexist: pss 2x[128,2,512] + po 2x[128,512] +
    shared proj/outproj pool 2x[128,512] = exactly 8 banks.
  - V tiles and the output projection of region r-1 are interleaved into
    the attention units (scores -> exp -> attnV lagged by two k-tiles so
    PE never waits on ACT).
  - All PSUM evacuations run on DVE (tensor_scalar_add fuses the bias);
    ACT does exp exclusively; softmax normalization on DVE+Pool off the
    critical path.

All matmuls run as float32r (fp32 data, bf16-rate PE path, fp32 accumulate).
fp8 variants were measured and rejected: attention output here is a near-
cancelling sum over random-sign V, so per-element weight quantization noise
(~3.6% for e4m3) passes straight to the output without averaging.
"""

import time

import numpy as np

import concourse.mybir as mybir
import concourse.tile as tile
from concourse import bacc
from concourse.bass_utils import run_bass_kernel_spmd

F32 = mybir.dt.float32
F32R = mybir.dt.float32r
AF = mybir.ActivationFunctionType

# Problem shapes (hardcoded per contest contract)
B, S, E, H, D = 2, 2048, 1024, 16, 64
NCORES = 8
HPC = H // (NCORES // B)   # heads per core = 4
NP = HPC // 2              # head-pairs per core = 2
DH = HPC * D               # head dims per core = 256
P = 128                    # partitions
SC = 512                   # projection chunk / q-region width
KO = E // P                # contraction tiles for projections = 8
NSC = S // SC              # token chunks = 4
KI = S // P                # k tiles = 16
NG = 3 * DH // P           # projection column groups = 6
VW = 2 * (D + 1)           # v_aug row width for 2 heads = 130


def build_kernel(tc, xt, wkqvt, bkqv, wot, partial):
    nc = tc.nc
    xt_r = xt.rearrange("(ko p) s -> p ko s", p=P)          # [128, 8, 2048]
    wkqvt_r = wkqvt.rearrange("(ko p) n -> p ko n", p=P)    # [128, 8, 768]

    with (
        tc.tile_pool(name="persist", bufs=1) as persist,
        tc.tile_pool(name="stream", bufs=3) as stream,
        tc.tile_pool(name="small", bufs=2) as small,
        tc.tile_pool(name="pss", bufs=2, space="PSUM") as pss_pool,
        tc.tile_pool(name="psoa", bufs=1, space="PSUM") as psoa,
        tc.tile_pool(name="psmat", bufs=2, space="PSUM") as psmat,
    ):
        # ---- resident tensors / startup DMA order ----
        # Transfers serialize on the DMA queue, so issue in consumption
        # order: bias + v-bias row, x chunk0, Wk, Wq (k/q projections chase
        # these), x chunks 1-3, Wv (v tiles start ~when attention starts),
        # Wo (needed first at outproj of region 0, much later).
        wkqv_sb = persist.tile([P, KO, 3 * DH], F32R)       # [128, 8, 768]
        bias_sb = persist.tile([P, NG], F32)
        vbias_row = persist.tile([P, DH], F32)
        nc.sync.dma_start(bias_sb[:], bkqv.rearrange("(g p) o -> p (g o)", p=P))
        nc.sync.dma_start(vbias_row[0:1, :],
                          bkqv[2 * DH:3 * DH].rearrange("d o -> o d"))
        xts = [stream.tile([P, KO, SC], F32R, name=f"xt{c}", tag=f"xt{c}",
                           bufs=1) for c in range(NSC)]
        # per-ko interleave of x chunk0 + Wk so the first k accumulation
        # chain chases the DMA stream instead of waiting for whole tiles
        for ko in range(KO):
            nc.sync.dma_start(xts[0][:, ko, :], xt_r[:, ko, 0:SC])
            nc.sync.dma_start(wkqv_sb[:, ko, 0:DH], wkqvt_r[:, ko, 0:DH])
        nc.sync.dma_start(wkqv_sb[:, :, DH:2 * DH], wkqvt_r[:, :, DH:2 * DH])  # Wq
        nc.sync.dma_start(wkqv_sb[:, :, 2 * DH:], wkqvt_r[:, :, 2 * DH:])    # Wv
        for c in range(1, NSC):
            nc.sync.dma_start(xts[c][:], xt_r[:, :, c * SC:(c + 1) * SC])
        wot_sb = persist.tile([P, NP, E], F32R)
        nc.sync.dma_start(wot_sb[:], wot.rearrange("(j p) e -> p j e", p=P))

        kT_sb = persist.tile([P, NP, S], F32R)   # per pair: rows = 2 heads x 64
        qT_sb = persist.tile([P, NP, S], F32R)
        v_sb = persist.tile([P, NP * KI, VW], F32R)  # per k-tile: [1 vA | 1 vB]
        attnT_sb = persist.tile([P, NP, S], F32R)

        # ones columns for the softmax denominator (attnV row 0 per head);
        # ACT Identity with scale=0 writes the 1.0s (memset can't set f32r)
        for col in (0, D + 1):
            nc.scalar.activation(
                v_sb[:, :, col],
                xts[0][:, 0, 0:NP * KI],
                AF.Identity, bias=1.0, scale=0.0)
        # v bias broadcast to all token partitions: [128, 256]
        vbias_bc = persist.tile([P, DH], F32)
        nc.gpsimd.partition_broadcast(vbias_bc[:], vbias_row[0:1, :])
        # preload the Exp activation table while projections run
        scratch = small.tile([P, 1], F32, tag="scr")
        nc.scalar.activation(scratch[:], bias_sb[:, 0:1], AF.Exp, scale=1.0)

        # ---- projection helpers ----
        def proj_kq(c, g):
            """Group g (0,1 = K pairs; 2,3 = Q pairs) over token chunk c."""
            cs = slice(c * SC, (c + 1) * SC)
            ps = psmat.tile([P, SC], F32, tag="mat")
            for ko in range(KO):
                nc.tensor.matmul(
                    ps[:],
                    wkqv_sb[:, ko, g * P:(g + 1) * P],
                    xts[c][:, ko, :],
                    start=(ko == 0), stop=(ko == KO - 1),
                )
            dest = (kT_sb if g < 2 else qT_sb)[:, g % 2, cs]
            nc.vector.tensor_scalar_add(dest, ps[:], bias_sb[:, g:g + 1])

        def v_tile(j):
            """V for token tile j directly in [token, dim] layout:
            stationary x [128e,128s], moving Wv [128e,256d]."""
            c, t = divmod(j, SC // P)
            ps = psmat.tile([P, SC], F32, tag="mat")
            for ko in range(KO):
                nc.tensor.matmul(
                    ps[:, 0:DH],
                    xts[c][:, ko, t * P:(t + 1) * P],
                    wkqv_sb[:, ko, 2 * DH:3 * DH],
                    start=(ko == 0), stop=(ko == KO - 1),
                )
            for p in range(NP):
                dst = v_sb[:, p * KI + j, :].rearrange(
                    "p (h w) -> p h w", h=2)[:, :, 1:D + 1]
                src = ps[:, p * P:(p + 1) * P].rearrange(
                    "p (h w) -> p h w", h=2)
                vb = vbias_bc[:, p * P:(p + 1) * P].rearrange(
                    "p (h w) -> p h w", h=2)
                nc.vector.tensor_add(dst, src, vb)

        def outproj(r, stis=range(SC // P)):
            """Output projection of region r (attnT rows already normalized)."""
            for sti in stis:
                row = r * SC + sti * P
                ot = stream.tile([P, E], F32, tag="ot", bufs=2)
                for e in range(E // SC):
                    pp = psmat.tile([P, SC], F32, tag="mat")
                    for p2 in range(NP):
                        nc.tensor.matmul(
                            pp[:],
                            attnT_sb[:, p2, row:row + P],
                            wot_sb[:, p2, e * SC:(e + 1) * SC],
                            start=(p2 == 0), stop=(p2 == NP - 1),
                        )
                    nc.vector.tensor_copy(ot[:, e * SC:(e + 1) * SC], pp[:])
                nc.sync.dma_start(partial[row:row + P, :], ot[:])

        # ---- chunk-0 k/q projections; chunks 1-3 are fused into unit 0 ----
        for g in range(4):
            proj_kq(0, g)

        # ---- attention units with proj/v/outproj work interleaved ----
        def att_unit(r, p, hooks, final=False):
            rs = slice(r * SC, (r + 1) * SC)
            po = [psoa.tile([P, SC], F32, name=f"oa{h}", tag=f"oa{h}", bufs=1)
                  for h in range(2)]
            exs = {}

            # attnV lags scores by two k-tiles: PE never sits in the
            # exp -> attnV -> next-scores semaphore chain
            def attn_v(k):
                ex_k = exs.pop(k)
                for h in range(2):
                    nc.tensor.matmul(
                        po[h][0:D + 1, :],
                        v_sb[:, p * KI + k, h * (D + 1):(h + 1) * (D + 1)],
                        ex_k[:, h, :],
                        start=(k == 0), stop=(k == KI - 1),
                        skip_group_check=True,
                    )

            for ki in range(KI):
                for fn in hooks.pop(ki, ()):
                    fn()
                pss = pss_pool.tile([P, 2, SC], F32, tag="sc")
                for h in range(2):
                    hr = slice(h * D, (h + 1) * D)
                    nc.tensor.matmul(
                        pss[:, h, :],
                        kT_sb[hr, p, ki * P:(ki + 1) * P],
                        qT_sb[hr, p, rs],
                        start=True, stop=True,
                    )
                ex = stream.tile([P, 2, SC], F32R, tag="exp", bufs=4)
                nc.scalar.activation(ex[:], pss[:], AF.Exp,
                                     scale=1.0 / np.sqrt(D))
                exs[ki] = ex
                if ki >= 2:
                    attn_v(ki - 2)
            attn_v(KI - 2)
            attn_v(KI - 1)
            for fn in hooks.pop("post", ()):
                fn()
            if not final:
                # softmax normalization (DVE+Pool) off the critical path
                for h in range(2):
                    oa = small.tile([D + 1, SC], F32, tag="oa_sb")
                    nc.vector.tensor_copy(oa[:], po[h][0:D + 1, :])
                    recip = small.tile([1, SC], F32, tag="recip")
                    nc.vector.reciprocal(recip[:], oa[0:1, :])
                    bc = small.tile([D + 1, SC], F32, tag="bc")
                    nc.gpsimd.partition_broadcast(bc[:], recip[:])
                    nrm = small.tile([D + 1, SC], F32R, tag="nrm")
                    nc.vector.tensor_mul(nrm[:], oa[:], bc[:])
                    nc.sync.dma_start(attnT_sb[h * D:(h + 1) * D, p, rs],
                                      nrm[1:D + 1, :])
                return
            # final unit: interleave per-sti normalization (reading PSUM
            # directly) with the last region's output projection so the
            # tail is one 128-token chain instead of a full-region one
            for sti in range(SC // P):
                cs = slice(sti * P, (sti + 1) * P)
                row = r * SC + sti * P
                for h in range(2):
                    recip = small.tile([1, P], F32, tag="recip")
                    nc.vector.reciprocal(recip[:], po[h][0:1, cs])
                    bc = small.tile([D + 1, P], F32, tag="bc")
                    nc.gpsimd.partition_broadcast(bc[:], recip[:])
                    nrm = small.tile([D + 1, P], F32R, tag="nrm")
                    nc.vector.tensor_mul(nrm[:], po[h][0:D + 1, cs], bc[:])
                    nc.sync.dma_start(
                        attnT_sb[h * D:(h + 1) * D, p, row:row + P],
                        nrm[1:D + 1, :])
                ot = stream.tile([P, E], F32, tag="ot", bufs=2)
                for e in range(E // SC):
                    pp = psmat.tile([P, SC], F32, tag="mat")
                    for p2 in range(NP):
                        nc.tensor.matmul(
                            pp[:],
                            attnT_sb[:, p2, row:row + P],
                            wot_sb[:, p2, e * SC:(e + 1) * SC],
                            start=(p2 == 0), stop=(p2 == NP - 1),
                        )
                    nc.vector.tensor_copy(ot[:, e * SC:(e + 1) * SC], pp[:])
                    nc.sync.dma_start(partial[row:row + P,
                                              e * SC:(e + 1) * SC],
                                      ot[:, e * SC:(e + 1) * SC])

        for r in range(NSC):
            for p in range(NP):
                hooks = {}
                if r == 0 and p == 0:
                    # chunk c's k/q projections + v tiles fused in just
                    # before the first scores that depend on chunk c; v
                    # tiles 0-3 one-by-one ahead of their attnV
                    for j in range(4):
                        hooks[j] = [lambda j=j: v_tile(j)]
                    for c in range(1, NSC):
                        work = [lambda c=c, g=g: proj_kq(c, g)
                                for g in range(4)]
                        work += [lambda j=j: v_tile(j)
                                 for j in range(4 * c, 4 * c + 4)]
                        hooks[4 * c] = work
                elif r > 0:
                    # split region r-1's outproj between this region's two
                    # units so PE load stays balanced against ACT's exps
                    if p == 0:
                        hooks[4] = [lambda r=r: outproj(r - 1, range(0, 2))]
                    else:
                        hooks[4] = [lambda r=r: outproj(r - 1, range(2, 4))]
                att_unit(r, p, hooks,
                         final=(r == NSC - 1 and p == NP - 1))


def build_module(reps=1):
    """reps>1 replicates the kernel body (serialized by an all-engine
    barrier) inside one NEFF so per-kernel HW time can be measured as the
    slope vs reps — the per-execute dispatch floor cancels."""
    nc = bacc.Bacc("TRN2", target_bir_lowering=False, debug=False,
                   num_devices=NCORES)
    xt = nc.dram_tensor("xt", [E, S], F32R, kind="ExternalInput").ap()
    wkqvt = nc.dram_tensor("wkqvt", [E, 3 * DH], F32R, kind="ExternalInput").ap()
    bkqv = nc.dram_tensor("bkqv", [3 * DH, 1], F32, kind="ExternalInput").ap()
    wot = nc.dram_tensor("wot", [DH, E], F32R, kind="ExternalInput").ap()
    partial = nc.dram_tensor("partial", [S, E], F32, kind="ExternalOutput").ap()
    with tile.TileContext(nc) as tc:
        for r in range(reps):
            build_kernel(tc, xt, wkqvt, bkqv, wot, partial)
            if reps > 1 and r < reps - 1:
                tc.strict_bb_all_engine_barrier()
    nc.compile()
    return nc


def make_in_maps(x, Wq, bq, Wk, bk, Wv, bv, Wo, bo):
    xts = [np.ascontiguousarray(x[b].T).astype(np.float32) for b in range(B)]
    in_maps = []
    for c in range(NCORES):
        b, hg = divmod(c, NCORES // B)
        rows = slice(hg * DH, (hg + 1) * DH)
        wkqvt = np.ascontiguousarray(
            np.concatenate([Wk[rows], Wq[rows], Wv[rows]], axis=0).T
        ).astype(np.float32)
        bkqv = np.concatenate([bk[rows], bq[rows], bv[rows]]).reshape(3 * DH, 1)
        wot = np.ascontiguousarray(Wo[:, rows].T).astype(np.float32)
        in_maps.append({
            "xt": xts[b],
            "wkqvt": wkqvt,
            "bkqv": bkqv.astype(np.float32),
            "wot": wot,
        })
    return in_maps


_NC_CACHE = None


def kernel(x, Wq, bq, Wk, bk, Wv, bv, Wo, bo, _trace=False):
    global _NC_CACHE
    x = np.asarray(x)
    if _NC_CACHE is None:
        _NC_CACHE = build_module()
    nc = _NC_CACHE
    in_maps = make_in_maps(np.asarray(x), np.asarray(Wq), np.asarray(bq),
                           np.asarray(Wk), np.asarray(bk), np.asarray(Wv),
                           np.asarray(bv), np.asarray(Wo), np.asarray(bo))
    # transient NRT_EXEC_UNIT_UNRECOVERABLE flakes have been observed on this
    # fabric; a short-delay retry has always succeeded.
    last_err = None
    for attempt in range(3):
        try:
            res = run_bass_kernel_spmd(nc, in_maps, core_ids=list(range(NCORES)),
                                       trace=_trace)
            break
        except Exception as e:  # noqa: BLE001
            last_err = e
            time.sleep(10 * (attempt + 1))
    else:
        raise last_err
    partials = np.stack([res.results[c]["partial"] for c in range(NCORES)])
    out = (partials.reshape(B, NCORES // B, S, E).sum(axis=1, dtype=np.float64)
           + np.asarray(bo, dtype=np.float64))
    out = out.astype(np.float32).reshape(B, S, E)
    if _trace:
        return out, res
    return out


# revision 18
# speedup vs baseline: 16.6231x; 1.0134x over previous
"""Trainium2 Bass kernel for nn_MultiHeadAttention (B=2, S=2048, E=1024, H=16, D=64).

Sharding: 8 NeuronCores = 2 batches x 4 head-groups (data + tensor parallel).
Each core computes, for its batch b and its 4 heads: q/k/v projections,
attention with softmax in transposed-score space (denominator via an extra
ones-column appended to V), and its partial of the output projection.  Host
sums the 4 partials per batch and adds the output bias.

Schedule (single fused pipeline; PE is the binding engine at ~167us of
matmul work, ACT ~137us of exp hides under it):
  - DMA order: bias, x chunk0, Wk, Wq, x chunks 1-3, Wv, Wo.  K/Q
    projections chase the x chunks so kT/qT complete as soon as the input
    lands; exp (ACT) starts right after.
  - V is projected DIRECTLY in [token, dim] layout (stationary x-tile,
    moving Wv) into PSUM, so no PE transposes / DVE repacking are needed
    and all PSUM pools coexist: pss 2x[128,2,512] + po 2x[128,512] +
    shared proj/outproj pool 2x[128,512] = exactly 8 banks.
  - V tiles and the output projection of region r-1 are interleaved into
    the attention units (scores -> exp -> attnV lagged by two k-tiles so
    PE never waits on ACT).
  - All PSUM evacuations run on DVE (tensor_scalar_add fuses the bias);
    ACT does exp exclusively; softmax normalization on DVE+Pool off the
    critical path.

All matmuls run as float32r (fp32 data, bf16-rate PE path, fp32 accumulate).
fp8 variants were measured and rejected: attention output here is a near-
cancelling sum over random-sign V, so per-element weight quantization noise
(~3.6% for e4m3) passes straight to the output without averaging.
"""

import time

import numpy as np

import concourse.mybir as mybir
import concourse.tile as tile
from concourse import bacc
from concourse.bass_utils import run_bass_kernel_spmd

F32 = mybir.dt.float32
F32R = mybir.dt.float32r
AF = mybir.ActivationFunctionType

# Problem shapes (hardcoded per contest contract)
B, S, E, H, D = 2, 2048, 1024, 16, 64
NCORES = 8
HPC = H // (NCORES // B)   # heads per core = 4
NP = HPC // 2              # head-pairs per core = 2
DH = HPC * D               # head dims per core = 256
P = 128                    # partitions
SC = 512                   # projection chunk / q-region width
KO = E // P                # contraction tiles for projections = 8
NSC = S // SC              # token chunks = 4
KI = S // P                # k tiles = 16
NG = 3 * DH // P           # projection column groups = 6
VW = 2 * (D + 1)           # v_aug row width for 2 heads = 130


def build_kernel(tc, xt, wkqvt, bkqv, wot, partial):
    nc = tc.nc
    xt_r = xt.rearrange("(ko p) s -> p ko s", p=P)          # [128, 8, 2048]
    wkqvt_r = wkqvt.rearrange("(ko p) n -> p ko n", p=P)    # [128, 8, 768]

    with (
        tc.tile_pool(name="persist", bufs=1) as persist,
        tc.tile_pool(name="stream", bufs=3) as stream,
        tc.tile_pool(name="small", bufs=2) as small,
        tc.tile_pool(name="pss", bufs=2, space="PSUM") as pss_pool,
        tc.tile_pool(name="psoa", bufs=1, space="PSUM") as psoa,
        tc.tile_pool(name="psmat", bufs=2, space="PSUM") as psmat,
    ):
        # ---- resident tensors / startup DMA order ----
        # Transfers serialize on the DMA queue, so issue in consumption
        # order: bias + v-bias row, x chunk0, Wk, Wq (k/q projections chase
        # these), x chunks 1-3, Wv (v tiles start ~when attention starts),
        # Wo (needed first at outproj of region 0, much later).
        wkqv_sb = persist.tile([P, KO, 3 * DH], F32R)       # [128, 8, 768]
        bias_sb = persist.tile([P, NG], F32)
        vbias_row = persist.tile([P, DH], F32)
        nc.sync.dma_start(bias_sb[:], bkqv.rearrange("(g p) o -> p (g o)", p=P))
        nc.sync.dma_start(vbias_row[0:1, :],
                          bkqv[2 * DH:3 * DH].rearrange("d o -> o d"))
        xts = [stream.tile([P, KO, SC], F32R, name=f"xt{c}", tag=f"xt{c}",
                           bufs=1) for c in range(NSC)]
        # per-ko interleave of x chunk0 + Wk + Wq so the first k/q
        # accumulation chains chase the DMA stream instead of whole tiles
        for ko in range(KO):
            nc.sync.dma_start(xts[0][:, ko, :], xt_r[:, ko, 0:SC])
            nc.sync.dma_start(wkqv_sb[:, ko, 0:2 * DH],
                              wkqvt_r[:, ko, 0:2 * DH])
        nc.sync.dma_start(wkqv_sb[:, :, 2 * DH:], wkqvt_r[:, :, 2 * DH:])    # Wv
        for c in range(1, NSC):
            nc.sync.dma_start(xts[c][:], xt_r[:, :, c * SC:(c + 1) * SC])
        wot_sb = persist.tile([P, NP, E], F32R)
        nc.sync.dma_start(wot_sb[:], wot.rearrange("(j p) e -> p j e", p=P))

        kT_sb = persist.tile([P, NP, S], F32R)   # per pair: rows = 2 heads x 64
        qT_sb = persist.tile([P, NP, S], F32R)
        v_sb = persist.tile([P, NP * KI, VW], F32R)  # per k-tile: [1 vA | 1 vB]
        attnT_sb = persist.tile([P, NP, S], F32R)

        # ones columns for the softmax denominator (attnV row 0 per head);
        # ACT Identity with scale=0 writes the 1.0s (memset can't set f32r)
        for col in (0, D + 1):
            nc.scalar.activation(
                v_sb[:, :, col],
                xts[0][:, 0, 0:NP * KI],
                AF.Identity, bias=1.0, scale=0.0)
        # v bias broadcast to all token partitions: [128, 256]
        vbias_bc = persist.tile([P, DH], F32)
        nc.gpsimd.partition_broadcast(vbias_bc[:], vbias_row[0:1, :])
        # preload the Exp activation table while projections run
        scratch = small.tile([P, 1], F32, tag="scr")
        nc.scalar.activation(scratch[:], bias_sb[:, 0:1], AF.Exp, scale=1.0)

        # ---- projection helpers ----
        def proj_kq(c, g):
            """Group g (0,1 = K pairs; 2,3 = Q pairs) over token chunk c."""
            cs = slice(c * SC, (c + 1) * SC)
            ps = psmat.tile([P, SC], F32, tag="mat")
            for ko in range(KO):
                nc.tensor.matmul(
                    ps[:],
                    wkqv_sb[:, ko, g * P:(g + 1) * P],
                    xts[c][:, ko, :],
                    start=(ko == 0), stop=(ko == KO - 1),
                )
            dest = (kT_sb if g < 2 else qT_sb)[:, g % 2, cs]
            nc.vector.tensor_scalar_add(dest, ps[:], bias_sb[:, g:g + 1])

        def v_tile(j):
            """V for token tile j directly in [token, dim] layout:
            stationary x [128e,128s], moving Wv [128e,256d]."""
            c, t = divmod(j, SC // P)
            ps = psmat.tile([P, SC], F32, tag="mat")
            for ko in range(KO):
                nc.tensor.matmul(
                    ps[:, 0:DH],
                    xts[c][:, ko, t * P:(t + 1) * P],
                    wkqv_sb[:, ko, 2 * DH:3 * DH],
                    start=(ko == 0), stop=(ko == KO - 1),
                )
            for p in range(NP):
                dst = v_sb[:, p * KI + j, :].rearrange(
                    "p (h w) -> p h w", h=2)[:, :, 1:D + 1]
                src = ps[:, p * P:(p + 1) * P].rearrange(
                    "p (h w) -> p h w", h=2)
                vb = vbias_bc[:, p * P:(p + 1) * P].rearrange(
                    "p (h w) -> p h w", h=2)
                nc.vector.tensor_add(dst, src, vb)

        def outproj(r, stis=range(SC // P)):
            """Output projection of region r (attnT rows already normalized)."""
            for sti in stis:
                row = r * SC + sti * P
                ot = stream.tile([P, E], F32, tag="ot", bufs=2)
                for e in range(E // SC):
                    pp = psmat.tile([P, SC], F32, tag="mat")
                    for p2 in range(NP):
                        nc.tensor.matmul(
                            pp[:],
                            attnT_sb[:, p2, row:row + P],
                            wot_sb[:, p2, e * SC:(e + 1) * SC],
                            start=(p2 == 0), stop=(p2 == NP - 1),
                        )
                    nc.vector.tensor_copy(ot[:, e * SC:(e + 1) * SC], pp[:])
                nc.sync.dma_start(partial[row:row + P, :], ot[:])

        # ---- chunk-0 k/q projections; chunks 1-3 are fused into unit 0 ----
        for g in range(4):
            proj_kq(0, g)

        # ---- attention units with proj/v/outproj work interleaved ----
        def att_unit(r, p, hooks, final=False, q0=0, W=SC):
            rs = slice(r * SC + q0, r * SC + q0 + W)
            po = [psoa.tile([P, SC], F32, name=f"oa{h}", tag=f"oa{h}", bufs=1)
                  for h in range(2)]
            exs = {}

            # attnV lags scores by two k-tiles: PE never sits in the
            # exp -> attnV -> next-scores semaphore chain
            def attn_v(k):
                ex_k = exs.pop(k)
                for h in range(2):
                    nc.tensor.matmul(
                        po[h][0:D + 1, 0:W],
                        v_sb[:, p * KI + k, h * (D + 1):(h + 1) * (D + 1)],
                        ex_k[:, h, 0:W],
                        start=(k == 0), stop=(k == KI - 1),
                        skip_group_check=True,
                    )

            for ki in range(KI):
                for fn in hooks.pop(ki, ()):
                    fn()
                pss = pss_pool.tile([P, 2, SC], F32, tag="sc")
                for h in range(2):
                    hr = slice(h * D, (h + 1) * D)
                    nc.tensor.matmul(
                        pss[:, h, 0:W],
                        kT_sb[hr, p, ki * P:(ki + 1) * P],
                        qT_sb[hr, p, rs],
                        start=True, stop=True,
                    )
                ex = stream.tile([P, 2, SC], F32R, tag="exp", bufs=4)
                nc.scalar.activation(ex[:, :, 0:W], pss[:, :, 0:W], AF.Exp,
                                     scale=1.0 / np.sqrt(D))
                exs[ki] = ex
                if ki >= 2:
                    attn_v(ki - 2)
            attn_v(KI - 2)
            attn_v(KI - 1)
            for fn in hooks.pop("post", ()):
                fn()
            if not final:
                # softmax normalization (DVE+Pool) off the critical path
                for h in range(2):
                    oa = small.tile([D + 1, SC], F32, tag="oa_sb")
                    nc.vector.tensor_copy(oa[:, 0:W], po[h][0:D + 1, 0:W])
                    recip = small.tile([1, SC], F32, tag="recip")
                    nc.vector.reciprocal(recip[:, 0:W], oa[0:1, 0:W])
                    bc = small.tile([D + 1, SC], F32, tag="bc")
                    nc.gpsimd.partition_broadcast(bc[:, 0:W], recip[:, 0:W])
                    nrm = small.tile([D + 1, SC], F32R, tag="nrm")
                    nc.vector.tensor_mul(nrm[:, 0:W], oa[:, 0:W], bc[:, 0:W])
                    nc.sync.dma_start(attnT_sb[h * D:(h + 1) * D, p, rs],
                                      nrm[1:D + 1, 0:W])
                return
            # final unit: interleave per-sti normalization (reading PSUM
            # directly) with the last region's output projection so the
            # tail is one 128-token chain instead of a full-region one
            for sti in range(W // P):
                cs = slice(sti * P, (sti + 1) * P)
                row = r * SC + q0 + sti * P
                for h in range(2):
                    recip = small.tile([1, P], F32, tag="recip")
                    nc.vector.reciprocal(recip[:], po[h][0:1, cs])
                    bc = small.tile([D + 1, P], F32, tag="bc")
                    nc.gpsimd.partition_broadcast(bc[:], recip[:])
                    nrm = small.tile([D + 1, P], F32R, tag="nrm")
                    nc.vector.tensor_mul(nrm[:], po[h][0:D + 1, cs], bc[:])
                    nc.sync.dma_start(
                        attnT_sb[h * D:(h + 1) * D, p, row:row + P],
                        nrm[1:D + 1, :])
                ot = stream.tile([P, E], F32, tag="ot", bufs=2)
                for e in range(E // SC):
                    pp = psmat.tile([P, SC], F32, tag="mat")
                    for p2 in range(NP):
                        nc.tensor.matmul(
                            pp[:],
                            attnT_sb[:, p2, row:row + P],
                            wot_sb[:, p2, e * SC:(e + 1) * SC],
                            start=(p2 == 0), stop=(p2 == NP - 1),
                        )
                    nc.vector.tensor_copy(ot[:, e * SC:(e + 1) * SC], pp[:])
                    nc.sync.dma_start(partial[row:row + P,
                                              e * SC:(e + 1) * SC],
                                      ot[:, e * SC:(e + 1) * SC])

        for r in range(NSC):
            for p in range(NP):
                hooks = {}
                if r == 0 and p == 0:
                    # chunk c's k/q projections + v tiles fused in just
                    # before the first scores that depend on chunk c; v
                    # tiles 0-3 one-by-one ahead of their attnV
                    for j in range(4):
                        hooks[j] = [lambda j=j: v_tile(j)]
                    for c in range(1, NSC):
                        work = [lambda c=c, g=g: proj_kq(c, g)
                                for g in range(4)]
                        work += [lambda j=j: v_tile(j)
                                 for j in range(4 * c, 4 * c + 4)]
                        hooks[4 * c] = work
                elif r > 0:
                    # split region r-1's outproj between this region's two
                    # units so PE load stays balanced against ACT's exps
                    if p == 0:
                        hooks[4] = [lambda r=r: outproj(r - 1, range(0, 2))]
                    else:
                        hooks[4] = [lambda r=r: outproj(r - 1, range(2, 4))]
                if r == NSC - 1 and p == NP - 1:
                    # last unit runs as two 256-wide halves: half A's norm
                    # and outproj hide under half B; only half B's short
                    # per-sti chains remain exposed at the very end
                    att_unit(r, p, hooks, q0=0, W=SC // 2)
                    hooksB = {4: [lambda: outproj(r, range(0, 2))]}
                    att_unit(r, p, hooksB, final=True, q0=SC // 2, W=SC // 2)
                else:
                    att_unit(r, p, hooks)


def build_module(reps=1):
    """reps>1 replicates the kernel body (serialized by an all-engine
    barrier) inside one NEFF so per-kernel HW time can be measured as the
    slope vs reps — the per-execute dispatch floor cancels."""
    nc = bacc.Bacc("TRN2", target_bir_lowering=False, debug=False,
                   num_devices=NCORES)
    xt = nc.dram_tensor("xt", [E, S], F32R, kind="ExternalInput").ap()
    wkqvt = nc.dram_tensor("wkqvt", [E, 3 * DH], F32R, kind="ExternalInput").ap()
    bkqv = nc.dram_tensor("bkqv", [3 * DH, 1], F32, kind="ExternalInput").ap()
    wot = nc.dram_tensor("wot", [DH, E], F32R, kind="ExternalInput").ap()
    partial = nc.dram_tensor("partial", [S, E], F32, kind="ExternalOutput").ap()
    with tile.TileContext(nc) as tc:
        for r in range(reps):
            build_kernel(tc, xt, wkqvt, bkqv, wot, partial)
            if reps > 1 and r < reps - 1:
                tc.strict_bb_all_engine_barrier()
    nc.compile()
    return nc


def make_in_maps(x, Wq, bq, Wk, bk, Wv, bv, Wo, bo):
    xts = [np.ascontiguousarray(x[b].T).astype(np.float32) for b in range(B)]
    in_maps = []
    for c in range(NCORES):
        b, hg = divmod(c, NCORES // B)
        rows = slice(hg * DH, (hg + 1) * DH)
        wkqvt = np.ascontiguousarray(
            np.concatenate([Wk[rows], Wq[rows], Wv[rows]], axis=0).T
        ).astype(np.float32)
        bkqv = np.concatenate([bk[rows], bq[rows], bv[rows]]).reshape(3 * DH, 1)
        wot = np.ascontiguousarray(Wo[:, rows].T).astype(np.float32)
        in_maps.append({
            "xt": xts[b],
            "wkqvt": wkqvt,
            "bkqv": bkqv.astype(np.float32),
            "wot": wot,
        })
    return in_maps


_NC_CACHE = None


def kernel(x, Wq, bq, Wk, bk, Wv, bv, Wo, bo, _trace=False):
    global _NC_CACHE
    x = np.asarray(x)
    if _NC_CACHE is None:
        _NC_CACHE = build_module()
    nc = _NC_CACHE
    in_maps = make_in_maps(np.asarray(x), np.asarray(Wq), np.asarray(bq),
                           np.asarray(Wk), np.asarray(bk), np.asarray(Wv),
                           np.asarray(bv), np.asarray(Wo), np.asarray(bo))
    # transient NRT_EXEC_UNIT_UNRECOVERABLE flakes have been observed on this
    # fabric; a short-delay retry has always succeeded.
    last_err = None
    for attempt in range(3):
        try:
            res = run_bass_kernel_spmd(nc, in_maps, core_ids=list(range(NCORES)),
                                       trace=_trace)
            break
        except Exception as e:  # noqa: BLE001
            last_err = e
            time.sleep(10 * (attempt + 1))
    else:
        raise last_err
    partials = np.stack([res.results[c]["partial"] for c in range(NCORES)])
    out = (partials.reshape(B, NCORES // B, S, E).sum(axis=1, dtype=np.float64)
           + np.asarray(bo, dtype=np.float64))
    out = out.astype(np.float32).reshape(B, S, E)
    if _trace:
        return out, res
    return out
